# revision 13
# baseline (speedup 1.0000x reference)
"""Per-image 256-bin luma-histogram entropy on Trainium2 (Bass, 8-core SPMD), v2.

Input  x: (32, 3, 512, 512) fp32 RGB in [0,1]
Output   : (32,) fp32 entropy scores

Sharding: pure data parallel - batch split 4 images per NeuronCore.

v2 changes vs v1 (all engines rebalanced):
  - Luma moved off DVE onto TensorE: per 512-col chunk, 3 accumulating
    float32r matmuls with scaled-identity stationary operands compute
    psum_y = 255*(0.299 R + 0.587 G + 0.114 B) directly in PSUM.
  - Rounding via convert-truncation: u16 = int16(psum_y + 0.5) in ONE
    tensor_scalar (fp32->int16 convert truncates toward zero; y>=0).
  - Lo planes fused: (u & 15) >= s as a single tensor_scalar 2-op chain
    (no separate vlo tensor).
  - ACT planes read u16 (exact integers) as Sign(u - 16t + 0.5); plane
    split DVE hi t=1..7 + lo s=1..15 (22 ops), ACT hi t=8..15 (8 ops).

Per-half engine budget (cost model): DVE 2x658+22x327 = 8.5us,
ACT 8x1038 = 8.3us, PE 128x53 + 6x213 = 8.1us.

Engine sync: same-engine RAW/WAR needs explicit sem edges. Each DVE op
incs exactly one sem: sem_v by default, or its cross-engine signal sem.
"""

from contextlib import ExitStack

import numpy as np

N_IMG = 4  # images per core
N_CORES = 8
H = 512
W = 512
P = 128  # SBUF partitions
HALF = 1024  # pixel columns per half-image ([128, 1024] = 131072 px)
CHUNK = 512  # luma matmul chunk (psum bank)
NPIX = H * W  # pixels per image
EPS = 1e-8
LN2 = 0.6931471805599453
NHALF = N_IMG * 2
NGRP = HALF // 8  # 8-column matmul groups per half (128 cols each op)

W255 = [float(np.float64(w) * 255.0) for w in (0.299, 0.587, 0.114)]

# plane split between engines (hi t=1..15, lo s=1..15; t=0/s=0 are memset
# ones planes). ACT: hi as Sign(+-1); POOL (GpSimd): highest lo planes;
# DVE: the rest.
ACT_HI = tuple(range(9, 16))  # planes computed on ScalarE as sign (+-1)
DVE_HI = tuple(range(1, 9))  # planes computed on DVE as is_ge ({0,1})
POOL_LO = tuple(range(12, 16))  # lo planes on GpSimd (is_ge on vlo)
DVE_LO = tuple(s for s in range(1, 16) if s not in POOL_LO)


def build_bass(reps=1):
    """Build the per-core Bass program. reps>1 repeats the whole pipeline
    (for marginal-cost timing); semaphore thresholds are offset per rep."""
    import concourse.bass as bass
    import concourse.mybir as mybir

    f32 = mybir.dt.float32
    f32r = mybir.dt.float32r
    # float32r is only safe for the luma identity matmuls; the selector /
    # score matmuls produce garbage on real silicon with f32r operands
    # (fp32 costs the same 4 cycles/row at FD=128 anyway).
    f32_luma = f32r
    f32_sel = f32
    bf16 = mybir.dt.bfloat16
    i16 = mybir.dt.int16
    Alu = mybir.AluOpType
    Act = mybir.ActivationFunctionType
    Axis = mybir.AxisListType

    nc = bass.Bass()

    x_t = nc.dram_tensor("x", [N_IMG, 3, H, W], f32_luma, kind="ExternalInput")
    sel_t = nc.dram_tensor("sel", [P, 16], f32_sel, kind="ExternalInput")
    mask_t = nc.dram_tensor("mask", [P, P], f32, kind="ExternalInput")
    ones_t = nc.dram_tensor("ones16", [16, 2], f32_sel, kind="ExternalInput")
    id3_t = nc.dram_tensor("id3", [P, 3 * P], f32_luma, kind="ExternalInput")
    out_t = nc.dram_tensor("out", [N_IMG], f32, kind="ExternalOutput")

    ctx = ExitStack()
    with ctx:
        # SBUF
        rgb = [
            ctx.enter_context(nc.sbuf_tensor(f"rgb{n}", [P, 3 * HALF], f32_luma))
            for n in range(3)
        ]
        u16 = [
            ctx.enter_context(nc.sbuf_tensor(f"u16_{n}", [P, HALF], i16))
            for n in range(2)
        ]
        vlo = [
            ctx.enter_context(nc.sbuf_tensor(f"vlo_{n}", [P, HALF], i16))
            for n in range(2)
        ]
        hi_b = [
            ctx.enter_context(nc.sbuf_tensor(f"hi{n}", [P, 16 * HALF], bf16))
            for n in range(2)
        ]
        lo_b = [
            ctx.enter_context(nc.sbuf_tensor(f"lo{n}", [P, 16 * HALF], bf16))
            for n in range(2)
        ]
        sel_sb = ctx.enter_context(nc.sbuf_tensor("sel_sb", [P, 16], f32_sel))
        mask_sb = ctx.enter_context(nc.sbuf_tensor("mask_sb", [P, P], f32))
        ones_sb = ctx.enter_context(nc.sbuf_tensor("ones_sb", [16, 2], f32_sel))
        id3_sb = ctx.enter_context(nc.sbuf_tensor("id3_sb", [P, 3 * P], f32_luma))
        p_sb = [
            ctx.enter_context(nc.sbuf_tensor(f"p_sb{n}", [P, P], f32_sel))
            for n in range(2)
        ]
        mm4 = ctx.enter_context(nc.sbuf_tensor("mm4", [16, 16], f32))
        hist4 = ctx.enter_context(nc.sbuf_tensor("hist4", [16, 16 * N_IMG], f32))
        ln4 = ctx.enter_context(nc.sbuf_tensor("ln4", [16, 16 * N_IMG], f32))
        e4 = ctx.enter_context(nc.sbuf_tensor("e4", [16, 16 * N_IMG], f32))
        part = ctx.enter_context(nc.sbuf_tensor("part", [16, N_IMG], f32_sel))
        score_sb = ctx.enter_context(nc.sbuf_tensor("score_sb", [N_IMG, 1], f32))
        warm = ctx.enter_context(nc.sbuf_tensor("warm", [1, 2], f32))
        eps_sb = ctx.enter_context(nc.sbuf_tensor("eps_sb", [16, 1], f32))
        bias_sb = ctx.enter_context(
            nc.sbuf_tensor("bias_sb", [P, len(ACT_HI)], f32)
        )

        # PSUM (8 banks): hist split even/odd images over 2 banks so reading
        # image i never overlaps an open accumulation group in the same bank;
        # 3 rotating luma chunk banks (luma runs ~2 halves ahead of hist).
        psum_h = [
            ctx.enter_context(nc.psum_tensor(f"psum_h{n}", [P, 2 * P], f32))
            for n in range(2)
        ]
        psum_y = [
            ctx.enter_context(nc.psum_tensor(f"psum_y{q}", [P, CHUNK], f32))
            for q in range(4)
        ]
        psum_oo = ctx.enter_context(nc.psum_tensor("psum_oo", [16, 260], f32))
        psum_o = [psum_oo[:, 0:P], psum_oo[:, P : 2 * P]]
        psum_s = psum_oo[0:N_IMG, 2 * P : 2 * P + 2]
        psum_s0 = psum_oo[0:N_IMG, 2 * P : 2 * P + 1]

        # semaphores
        sem_dma = [
            ctx.enter_context(nc.semaphore(f"dma_in{n}")) for n in range(3)
        ]
        sem_cdma = ctx.enter_context(nc.semaphore("const_dma"))
        sem_id3 = ctx.enter_context(nc.semaphore("id3_dma"))
        sem_lu = ctx.enter_context(nc.semaphore("luma"))
        sem_u16 = ctx.enter_context(nc.semaphore("u16done"))
        sem_pl = ctx.enter_context(nc.semaphore("planes"))
        sem_pla = ctx.enter_context(nc.semaphore("planes_act"))
        sem_plp = ctx.enter_context(nc.semaphore("planes_pool"))
        sem_vlo = ctx.enter_context(nc.semaphore("vlo"))
        sem_peh = ctx.enter_context(nc.semaphore("pe_half"))
        sem_psb = ctx.enter_context(nc.semaphore("psb"))
        sem_smm = ctx.enter_context(nc.semaphore("selmm"))
        sem_red = ctx.enter_context(nc.semaphore("red"))
        sem_ln = ctx.enter_context(nc.semaphore("ln"))
        sem_part = ctx.enter_context(nc.semaphore("part"))
        sem_sm = ctx.enter_context(nc.semaphore("scoremm"))
        sem_sc = ctx.enter_context(nc.semaphore("score"))
        sem_out = ctx.enter_context(nc.semaphore("out_dma"))
        sem_v = ctx.enter_context(nc.semaphore("dve_chain"))
        sem_wm = ctx.enter_context(nc.semaphore("warm"))

        def x_half_ap(i, c, h):
            # [512,512] -> [128, 2048] (4 consecutive rows per partition), half h
            a = x_t[i, c].rearrange("(p r) w -> p (r w)", r=4)
            return a[:, h * HALF : (h + 1) * HALF]

        def plane(buf, t):
            # blocked plane slot t of a hi/lo buffer: [128, NGRP, 8] strided
            return buf[:].rearrange("p (g j c) -> p g j c", j=16, c=8)[:, :, t, :]

        TOT = reps * NHALF

        with nc.Block() as block:

            @block.sync
            def _(sync):
                # id3 first (warm-up matmuls and luma need only it); the
                # other consts queue behind half 0's rgb so luma(0) starts
                # ~1.5us earlier. They are needed only from the h=3 tail.
                sync.dma_start(out=id3_sb[:], in_=id3_t[:]).then_inc(sem_id3, 16)
                for gh in range(TOT):
                    k = gh % NHALF
                    i, h = divmod(k, 2)
                    b = gh % 3
                    if gh >= 3:
                        # rgb[b] free once luma of half gh-3 has read it
                        sync.wait_ge(sem_lu, 2 * (gh - 2))
                    for c in range(3):
                        sync.dma_start(
                            out=rgb[b][:, c * HALF : (c + 1) * HALF],
                            in_=x_half_ap(i, c, h),
                        ).then_inc(sem_dma[b], 16)
                    if gh == 0:
                        sync.dma_start(out=sel_sb[:], in_=sel_t[:]).then_inc(
                            sem_cdma, 16
                        )
                        sync.dma_start(out=mask_sb[:], in_=mask_t[:]).then_inc(
                            sem_cdma, 16
                        )
                        sync.dma_start(out=ones_sb[:], in_=ones_t[:]).then_inc(
                            sem_cdma, 16
                        )
                sync.wait_ge(sem_sc, reps)
                sync.dma_start(out=out_t[:], in_=score_sb[:, 0:1]).then_inc(
                    sem_out, 16
                )
                sync.wait_ge(sem_out, 16)

            @block.vector
            def _(vector):
                vcnt = 0

                def vop(inst, sem=None, val=1, w=None):
                    nonlocal vcnt
                    if w is not None:
                        inst._wait_ge(w[0], w[1])
                    if sem is None:
                        inst.then_inc(sem_v, 1)
                        vcnt += 1
                    else:
                        inst.then_inc(sem, val)
                    return inst

                def vwait():
                    vector.wait_ge(sem_v, vcnt)

                vop(vector.memset(warm[:], 1.0), sem=sem_wm)
                vop(vector.memset(eps_sb[:], EPS))
                for n, t in enumerate(ACT_HI):
                    vop(vector.memset(bias_sb[:, n : n + 1], 0.5 - 16.0 * t))
                # one-time ones planes (t=0 / s=0); never rewritten
                for n in range(2):
                    vop(vector.memset(plane(hi_b[n], 0), 1.0))
                    vop(vector.memset(plane(lo_b[n], 0), 1.0))

                # ---- per-image tail, uniformly lagged behind the hist
                # stage (global image gi: TA at half 2gi+3, TB at 2gi+4;
                # per-rep entropy at 8r+11, score scale at 8r+13) ----
                def TA(gi):
                    i = gi % N_IMG
                    if gi >= 2:
                        vector.wait_ge(sem_smm, gi - 1)  # p_sb[gi%2] free
                    with nc.allow_low_precision(reason="f32r counts <= 2^15"):
                        inst = vector.tensor_tensor(
                            p_sb[gi % 2][:],
                            psum_h[i % 2][:, (i // 2) * P : (i // 2 + 1) * P],
                            mask_sb[:],
                            Alu.mult,
                        )
                    vop(inst, sem=sem_psb, w=(sem_peh, 2 * gi + 2))

                def TB(gi):
                    i = gi % N_IMG
                    r = gi // N_IMG
                    if r >= 1:
                        # Ln(r-1) must have read hist4 before overwrite
                        vector.wait_ge(sem_ln, r)
                    src = psum_o[gi % 2].rearrange("j (l c) -> j l c", c=8)
                    vwait()
                    vector.wait_ge(sem_red, gi)  # mm4 free (prior copy done)
                    vop(
                        vector.tensor_reduce(mm4[:], src, Axis.X, Alu.add),
                        w=(sem_smm, gi + 1),
                    )
                    vop(
                        vector.tensor_tensor(
                            hist4[:, 16 * i : 16 * i + 15],
                            mm4[:, 0:15],
                            mm4[:, 1:16],
                            Alu.subtract,
                        ),
                        w=(sem_v, vcnt),
                    )
                    vop(
                        vector.tensor_copy(
                            hist4[:, 16 * i + 15 : 16 * i + 16], mm4[:, 15:16]
                        ),
                        sem=sem_red,
                    )

                def dve_tail(h):
                    if h >= 3 and h % 2 == 1 and (h - 3) // 2 < N_IMG * reps:
                        if h == 3:
                            vector.wait_ge(sem_cdma, 48)  # consts loaded
                        TA((h - 3) // 2)
                    if h >= 4 and h % 2 == 0 and (h - 4) // 2 < N_IMG * reps:
                        TB((h - 4) // 2)
                    if h >= 11 and (h - 11) % 8 == 0 and (h - 11) // 8 < reps:
                        r = (h - 11) // 8
                        vwait()
                        vop(
                            vector.tensor_tensor(
                                e4[:], hist4[:], ln4[:], Alu.mult
                            ),
                            w=(sem_ln, r + 1),
                        )
                        with nc.allow_low_precision(
                            reason="f32r partial entropy sums"
                        ):
                            inst = vector.tensor_reduce(
                                part[:],
                                e4[:].rearrange("p (i l) -> p i l", i=N_IMG),
                                Axis.X,
                                Alu.add,
                            )
                        vop(inst, sem=sem_part, w=(sem_v, vcnt))
                    if h >= 13 and (h - 13) % 8 == 0 and (h - 13) // 8 < reps:
                        r = (h - 13) // 8
                        vop(
                            vector.tensor_scalar(
                                score_sb[:],
                                psum_s0,
                                -1.0 / (NPIX * LN2),
                                None,
                                Alu.mult,
                            ),
                            sem=sem_sc,
                            w=(sem_sm, r + 1),
                        )

                for gh in range(TOT):
                    r, k = divmod(gh, NHALF)
                    b = gh % 2
                    # u16 chunks (psum_y -> int16, +0.5 then trunc)
                    for q in range(2):
                        if gh >= 2 and q == 0:
                            # WAR: ACT+POOL planes of gh-2 done reading u16[b]
                            vector.wait_ge(sem_pla, gh - 1)
                            vector.wait_ge(sem_plp, gh - 1)
                        inst = vector.tensor_scalar(
                            u16[b][:, q * CHUNK : (q + 1) * CHUNK],
                            psum_y[(2 * gh + q) % 4][:],
                            0.5,
                            None,
                            Alu.add,
                        )
                        inst._wait_ge(sem_lu, 2 * gh + q + 1)
                        inst.then_inc(sem_u16, 1)
                    # vlo = u16 & 15 (same-engine RAW via sem_u16 wait);
                    # walrus rejects bitwise+arith chains, so lo planes read
                    # a precomputed nibble tensor with plain is_ge
                    inst = vector.tensor_scalar(
                        vlo[b][:], u16[b][:], 15, None, Alu.bitwise_and
                    )
                    inst._wait_ge(sem_u16, 2 * gh + 2)
                    inst.then_inc(sem_vlo, 1)
                    if gh >= 2:
                        vector.wait_ge(sem_peh, gh - 1)  # plane bufs b free
                    n_pl = len(DVE_HI) + len(DVE_LO)
                    n_done = 0
                    for t in DVE_HI:
                        n_done += 1
                        inst = vector.tensor_scalar(
                            plane(hi_b[b], t), u16[b][:], 16 * t, None, Alu.is_ge
                        )
                        vop(inst, sem=sem_pl if n_done == n_pl else None)
                    for s in DVE_LO:
                        n_done += 1
                        inst = vector.tensor_scalar(
                            plane(lo_b[b], s), vlo[b][:], s, None, Alu.is_ge
                        )
                        if n_done == len(DVE_HI) + 1:
                            inst._wait_ge(sem_vlo, gh + 1)  # same-eng RAW
                        vop(inst, sem=sem_pl if n_done == n_pl else None)

                    dve_tail(gh)
                for h in range(TOT, NHALF * reps + 6):
                    dve_tail(h)

            @block.tensor
            def _(tensor):
                def selmm(gi):
                    tensor.wait_ge(sem_psb, gi + 1)
                    if gi >= 1:
                        # psum_oo shared: prior TB must be fully done
                        tensor.wait_ge(sem_red, gi)
                    tensor.matmul(
                        psum_o[gi % 2],
                        lhsT=sel_sb[:],
                        rhs=p_sb[gi % 2][:],
                        start=True,
                        stop=True,
                    ).then_inc(sem_smm, 1)

                def pe_tail(ph):
                    if ph >= 3 and ph % 2 == 1 and (ph - 3) // 2 < N_IMG * reps:
                        selmm((ph - 3) // 2)
                    if ph >= 11 and (ph - 11) % 8 == 0 and (ph - 11) // 8 < reps:
                        r = (ph - 11) // 8
                        tensor.wait_ge(sem_part, r + 1)
                        if r >= 1:
                            tensor.wait_ge(sem_sc, r)  # psum_s free
                        tensor.matmul(
                            psum_s,
                            lhsT=part[:],
                            rhs=ones_sb[:],
                            start=True,
                            stop=True,
                        ).then_inc(sem_sm, 1)

                # warm-up matmuls: keep the PE HAM window busy through the
                # first DMA so the real stream starts at full clock
                tensor.wait_ge(sem_id3, 16)
                for _ in range(48):
                    tensor.matmul(
                        psum_o[0][:, 0:32],
                        lhsT=id3_sb[:, 0:16],
                        rhs=id3_sb[:, 0:32],
                        start=True,
                        stop=True,
                    )
                for it in range(TOT + 1):
                    # ---- luma chunks, two halves ahead of hist ----
                    if it == 0:
                        lumas = [0, 1] if TOT >= 2 else [0]
                    elif it + 1 <= TOT - 1:
                        lumas = [it + 1]
                    else:
                        lumas = []
                    for j in lumas:
                        b = j % 3
                        tensor.wait_ge(sem_dma[b], 48 * (j // 3 + 1))
                        for q in range(2):
                            cid = 2 * j + q
                            for c in range(3):
                                inst = tensor.matmul(
                                    psum_y[cid % 4][:],
                                    lhsT=id3_sb[:, c * P : (c + 1) * P],
                                    rhs=rgb[b][
                                        :,
                                        c * HALF + q * CHUNK : c * HALF
                                        + (q + 1) * CHUNK,
                                    ],
                                    start=(c == 0),
                                    stop=(c == 2),
                                )
                                if c == 0 and cid >= 4:
                                    # psum_y slot free: u16 chunk cid-4 done
                                    inst._wait_ge(sem_u16, cid - 3)
                                if c == 2:
                                    inst.then_inc(sem_lu, 1)

                    # ---- hist matmuls for half it-1 ----
                    if it >= 1:
                        ph = it - 1
                        rr, kk = divmod(ph, NHALF)
                        bb = ph % 2
                        i, h = divmod(kk, 2)
                        gi = rr * N_IMG + i
                        if h == 0 and gi >= 2:
                            # bank shared with image gi-2: its mask-mult must
                            # have read it before this group opens
                            tensor.wait_ge(sem_psb, gi - 1)
                        tensor.wait_ge(sem_pla, ph + 1)
                        tensor.wait_ge(sem_plp, ph + 1)
                        last = None
                        for g in range(NGRP):
                            last = tensor.matmul(
                                psum_h[i % 2][
                                    :, (i // 2) * P : (i // 2 + 1) * P
                                ],
                                lhsT=hi_b[bb][:, 128 * g : 128 * (g + 1)],
                                rhs=lo_b[bb][:, 128 * g : 128 * (g + 1)],
                                start=(h == 0 and g == 0),
                                stop=(h == 1 and g == NGRP - 1),
                            )
                            if g == 0:
                                last._wait_ge(sem_pl, ph + 1)
                        last.then_inc(sem_peh, 1)

                        pe_tail(ph)
                for ph in range(TOT, NHALF * reps + 6):
                    pe_tail(ph)

            @block.gpsimd
            def _(gpsimd):
                for gh in range(TOT):
                    b = gh % 2
                    if gh >= 2:
                        gpsimd.wait_ge(sem_peh, gh - 1)  # plane bufs b free
                    gpsimd.wait_ge(sem_vlo, gh + 1)  # vlo[b] ready
                    for n, s in enumerate(POOL_LO):
                        inst = gpsimd.tensor_scalar(
                            plane(lo_b[b], s), vlo[b][:], s, None, Alu.is_ge
                        )
                        if n == len(POOL_LO) - 1:
                            inst.then_inc(sem_plp, 1)

            @block.scalar
            def _(scalar):
                # warm up the Ln/Sign tables early
                scalar.wait_ge(sem_wm, 1)
                scalar.activation(warm[:], warm[:], Act.Ln, bias=1.0, scale=0.0)
                for gh in range(reps * NHALF):
                    r, k = divmod(gh, NHALF)
                    b = gh % 2
                    if gh >= 2:
                        scalar.wait_ge(sem_peh, gh - 1)  # plane bufs b free
                    scalar.wait_ge(sem_u16, 2 * gh + 2)  # u16[b] ready
                    for n, t in enumerate(ACT_HI):
                        inst = scalar.activation(
                            plane(hi_b[b], t),
                            u16[b][:],
                            Act.Sign,
                            bias=bias_sb[:, n : n + 1],
                            scale=1.0,
                        )
                        if n == len(ACT_HI) - 1:
                            inst.then_inc(sem_pla, 1)
                    # ---- per-rep Ln (rep r's hist4 complete at half 8r+10) ----
                    if gh >= 10 and (gh - 10) % 8 == 0:
                        rl = (gh - 10) // 8
                        scalar.wait_ge(sem_red, (rl + 1) * N_IMG)
                        scalar.activation(
                            ln4[:],
                            hist4[:],
                            Act.Ln,
                            bias=eps_sb[:],
                            scale=1.0 / NPIX,
                        ).then_inc(sem_ln, 1)
                # last rep's Ln lands past the half loop
                scalar.wait_ge(sem_red, reps * N_IMG)
                scalar.activation(
                    ln4[:],
                    hist4[:],
                    Act.Ln,
                    bias=eps_sb[:],
                    scale=1.0 / NPIX,
                ).then_inc(sem_ln, 1)

    return nc


_NC_CACHE = {}


def _get_nc(reps=1):
    if reps not in _NC_CACHE:
        _NC_CACHE[reps] = build_bass(reps)
    return _NC_CACHE[reps]


def consts():
    # psum row index m = t*8 + c (t = hi plane, c = col-in-group).
    # F[t, a] = f_t(a) over hi-nibble values a; sel bakes W = F^-1 so the
    # selector matmul yields true per-hi-value counts from the mixed family.
    F = np.zeros((16, 16), np.float64)
    F[0, :] = 1.0
    for t in range(1, 16):
        step = (np.arange(16) >= t).astype(np.float64)
        F[t, :] = 2.0 * step - 1.0 if t in ACT_HI else step
    Wr = np.linalg.inv(F)  # [j', t]
    assert np.abs(Wr @ F - np.eye(16)).max() < 1e-9
    sel = np.zeros((P, 16), np.float32)
    for k in range(P):
        sel[k, :] = Wr[:, k // 8]
    mask = np.zeros((P, P), np.float32)
    for k in range(P):
        mask[k, k % 8 :: 8] = 1.0
    ones16 = np.ones((16, 2), np.float32)
    id3 = np.zeros((P, 3 * P), np.float32)
    for c in range(3):
        id3[:, c * P : (c + 1) * P] = np.eye(P, dtype=np.float32) * np.float32(
            W255[c]
        )
    return sel, mask, ones16, id3


def make_in_maps(x):
    x = np.ascontiguousarray(np.asarray(x, dtype=np.float32))
    assert x.shape == (N_IMG * N_CORES, 3, H, W)
    sel, mask, ones16, id3 = consts()
    return [
        {
            "x": np.ascontiguousarray(x[N_IMG * i : N_IMG * (i + 1)]),
            "sel": sel,
            "mask": mask,
            "ones16": ones16,
            "id3": id3,
        }
        for i in range(N_CORES)
    ]


def kernel(x):
    from concourse.bass_utils import run_bass_kernel_spmd

    nc = _get_nc()
    in_maps = make_in_maps(x)
    res = run_bass_kernel_spmd(nc, in_maps, core_ids=list(range(N_CORES)))
    return np.concatenate([res.results[i]["out"] for i in range(N_CORES)])
